# revision 8
# baseline (speedup 1.0000x reference)
"""EntropyBottleneck forward (q_mode='noise') as a Trainium2 Bass kernel.

Math
----
reference computes, per channel c with tiny per-channel params (W_k, b_k, f_k):

    y    = x + noise
    v    = y flattened per channel
    L(v) = chain of FactorizeCell: u <- softplus(W_k) @ u + b_k,
           then u <- u + tanh(f_k) * tanh(u)   (for k < last)
    lower = L(v - 0.5); upper = L(v + 0.5)
    s     = -sign(lower + upper)
    lik   = max(|sigmoid(s*upper) - sigmoid(s*lower)|, 1e-9)

When every gate f_k == 0 (true for this module's initialization), the chain is
per-channel *affine*: L(v) = M_c * v + D_c, with M_c > 0 (product of softplus
matrices) and D_c foldable on the host from the (C,3,3)-at-most params.
Then with h = M_c/2:

    lik = sigmoid(h - |t|) - sigmoid(-h - |t|)        (sign trick folded)
        = 0.5 * (tanh((t + h)/2) - tanh((t - h)/2))   (tanh identity,
                                                       sign-free: always >= 0)
    where t = M_c * y + D_c.

The device kernel therefore does, per element (using the identity
0.5*(tanh(a/2) - tanh(b/2)) = sigmoid(a) - sigmoid(b), which folds the 0.5
into the activation):
    y   = x + noise                                  (vector engine)
    p   = sigmoid(M * y + (D + h))                   (scalar engine, fused affine)
    q   = sigmoid(M * y + (D - h))                   (scalar engine, fused affine)
    lik = max(p - q, 1e-9)                           (vector engine; subtract
                                                      writes bf16, clamp on bf16)

Sharding: data-parallel over batch, one batch element per NeuronCore (8 cores).
Per-core tensor (192, 4096) is viewed as (384, 2048): row r holds half of
channel r//2, so each SBUF partition maps to exactly one channel and the
per-channel coefficients become per-partition scale/bias operands.

Performance notes (from perfetto trace analysis):
  * The kernel is pure DMA: 3.15MB x + 3.15MB noise in, 3.15MB y (f32,
    bit-exact) + 1.57MB lik (bf16, rel err <= 2^-9) out = 11.0MB per core.
    Sustained single-ring DMA rate measured ~373 GB/s (HBM-per-NC wall), so
    the byte count is the dominant term; bf16 lik cuts ~4us.
  * All bulk transfers ride ONE HWDGE ring (SP='sync'): an A/B test that
    split loads/stores across the SP and ACT rings REGRESSED ~5us — the ACT
    ring starts ~3us later and per-engine packet efficiency drops ~25% when
    every SDMA engine round-robins between two rings (less sequential HBM
    access). Only the tiny param load rides the ACT ring.
  * In-flight transfers on the ring are NOT drained strictly FIFO: the
    SDMA engines split service across all queued transfers, so a store
    queued early steals bandwidth from still-running loads and delays the
    tail compute chain (measured: last load group +5us late). Stores are
    therefore deferred (gated on tile-1 adds) until loads are nearly done.
  * Each dma_start costs ~0.6-1.0us of descriptor-gen (DIRECT2D) on its
    issuing sequencer; tiles 0/1 load as single contiguous 1MB transfers.
    Tile 2 (the last to arrive) loads as 512KB halves so its add->sigmoid
    ->sub tail chain starts earlier.
  * The ~2.2us end-of-NEFF poll loop is a fixed epilogue (~51 polls
    regardless of semaphore count); semaphores are still consolidated into
    vector/scalar progress counters (exact: engines execute serially in
    program order) to keep the instruction streams short.
"""

import numpy as np

B, C, H, W = 8, 192, 64, 64
NCORES = 8
ROWS, COLS = 384, 2048  # (C, H*W) = (192, 4096) viewed as (384, 2048)
NT = ROWS // 128  # 3 row-tiles of 128 partitions
CH = 1024  # column chunk
NG = NT * (COLS // CH)  # 6 groups; g = 2*t + h

_CACHE: dict = {}


def _softplus64(x: np.ndarray) -> np.ndarray:
    x = x.astype(np.float64)
    return np.log1p(np.exp(-np.abs(x))) + np.maximum(x, 0.0)


def _fold_affine(ws, bs):
    """Compose the per-channel affine chain: L(v) = M*v + D. Returns (M, D) as (C,)."""
    M = np.ones((C, 1, 1), np.float64)
    D = np.zeros((C, 1, 1), np.float64)
    for Wk, bk in zip(ws, bs):
        spw = _softplus64(np.asarray(Wk))
        M = spw @ M
        D = spw @ D + np.asarray(bk, np.float64)
    return M[:, 0, 0], D[:, 0, 0]


def _numpy_fallback(x, noise, ws, bs, fs):
    """Exact replica of the reference chain for the general (gated) case."""
    x = np.asarray(x, np.float32)
    noise = np.asarray(noise, np.float32)
    y = x + noise
    v = y.transpose(1, 0, 2, 3).reshape(C, 1, -1).astype(np.float32)

    def logits(v):
        for i, (Wk, bk) in enumerate(zip(ws, bs)):
            spw = _softplus64(np.asarray(Wk)).astype(np.float32)
            v = np.einsum("coi,cin->con", spw, v) + np.asarray(bk, np.float32)
            if i < len(fs):
                v = v + np.tanh(np.asarray(fs[i], np.float32)) * np.tanh(v)
        return v

    lower = logits(v - 0.5)
    upper = logits(v + 0.5)
    sign = -np.sign(lower + upper)
    sig = lambda z: 1.0 / (1.0 + np.exp(-z, dtype=np.float32))
    lik = np.abs(sig(sign * upper) - sig(sign * lower))
    lik = np.maximum(lik, np.float32(1e-9))
    lik = lik.reshape(C, B, H, W).transpose(1, 0, 2, 3)
    return y, lik


def _build_program():
    """Hand-scheduled program: explicit per-engine instruction streams.

    sync   : ALL bulk DMA on the SP HWDGE ring — tile loads first (tiles 0/1
             as whole 1MB transfers, tile 2 as 512KB halves), then deferred
             y (f32) and lik (bf16) tile stores in compute-readiness order,
             then the final all-stores wait
    scalar : single param load (ACT ring), sigmoid pairs per [128, CH] chunk
    vector : adds per chunk; per-tile subtract (bf16 out) + clamp

    Cross-engine sync via two progress counters (vp: vector, sp: scalar) that
    each engine bumps in program order — exact because one engine executes
    serially — plus one completion semaphore per load group (full-group
    thresholds only: per-transfer DMA increments interleave across in-flight
    transfers, so prefix thresholds on a shared DMA semaphore are racy).
    """
    import concourse.bacc as bacc
    import concourse.mybir as mybir

    f32 = mybir.dt.float32
    bf16 = mybir.dt.bfloat16
    nc = bacc.Bacc("TRN2", target_bir_lowering=False, debug=False,
                   num_devices=NCORES)

    x_d = nc.dram_tensor("x", [ROWS, COLS], f32, kind="ExternalInput")
    n_d = nc.dram_tensor("noise", [ROWS, COLS], f32, kind="ExternalInput")
    p_d = nc.dram_tensor("prm", [128, 3 * NT], f32, kind="ExternalInput")
    y_d = nc.dram_tensor("y", [ROWS, COLS], f32, kind="ExternalOutput")
    l_d = nc.dram_tensor("lik", [ROWS, COLS], bf16, kind="ExternalOutput")

    Sigmoid = mybir.ActivationFunctionType.Sigmoid
    op_add = mybir.AluOpType.add
    op_sub = mybir.AluOpType.subtract
    op_max = mybir.AluOpType.max

    prm = nc.alloc_sbuf_tensor("prm_t", [128, 3 * NT], f32)
    xts = [nc.alloc_sbuf_tensor(f"xt{t}", [128, COLS], f32) for t in range(NT)]
    nts = [nc.alloc_sbuf_tensor(f"nt{t}", [128, COLS], f32) for t in range(NT)]
    yts = [nc.alloc_sbuf_tensor(f"yt{t}", [128, COLS], f32) for t in range(NT)]
    pts = [nc.alloc_sbuf_tensor(f"pt{t}", [128, COLS], f32) for t in range(NT)]
    qts = [nc.alloc_sbuf_tensor(f"qt{t}", [128, COLS], f32) for t in range(NT)]
    lts = [nc.alloc_sbuf_tensor(f"lt{t}", [128, COLS], bf16) for t in range(NT)]

    # Load-group completion sems: whole tiles 0/1, then tile-2 halves.
    ldt = [nc.alloc_semaphore(f"ldt{i}") for i in range(4)]
    ldp = nc.alloc_semaphore("ldp")  # param load
    vp = nc.alloc_semaphore("vp")  # vector progress (engine-ordered +1s)
    sp = nc.alloc_semaphore("sp")  # scalar-act progress (engine-ordered +1s)
    st = nc.alloc_semaphore("st")  # all store completions (6 x 16)

    # Vector program order & the vp value after each op:
    #   add0=1 add1=2 add2=3 add3=4 ts0=5 add4=6 add5=7 ts1=8 ts2=9
    VP_ADD = [1, 2, 3, 4, 6, 7]
    VP_TS = [5, 8, 9]
    VP_Y = [2, 4, 7]  # y tile t needs adds of chunks 2t, 2t+1
    # Which load sem gates the add of chunk g, and its threshold: tiles 0/1
    # arrive as one x + one noise transfer (2 x 16 incs); tile-2 halves too.
    ADD_GATE = [(0, 32), (0, 32), (1, 32), (1, 32), (2, 32), (3, 32)]

    def grc(g):
        t, h = divmod(g, 2)
        return t, slice(t * 128, (t + 1) * 128), slice(h * CH, (h + 1) * CH)

    with nc.Block(no_gpsimd_drain=True) as block:

        @block.sync
        def _(sync):
            for t in range(2):
                rows = slice(t * 128, (t + 1) * 128)
                sync.dma_start(xts[t][:], x_d[rows, :]).then_inc(ldt[t], 16)
                sync.dma_start(nts[t][:], n_d[rows, :]).then_inc(ldt[t], 16)
            rows2 = slice(2 * 128, 3 * 128)
            for h in range(2):
                cols = slice(h * CH, (h + 1) * CH)
                sync.dma_start(xts[2][:, cols], x_d[rows2, cols]).then_inc(ldt[2 + h], 16)
                sync.dma_start(nts[2][:, cols], n_d[rows2, cols]).then_inc(ldt[2 + h], 16)

            def y_store(t, vp_need):
                rows = slice(t * 128, (t + 1) * 128)
                sync.wait_ge(vp, vp_need)
                sync.dma_start(y_d[rows, :], yts[t][:]).then_inc(st, 16)

            def l_store(t):
                rows = slice(t * 128, (t + 1) * 128)
                sync.wait_ge(vp, VP_TS[t])
                sync.dma_start(l_d[rows, :], lts[t][:]).then_inc(st, 16)

            # Stores deferred to ~tile-1-adds-done so their packets don't
            # steal SDMA service from the still-draining loads; thresholds
            # are monotone so the FIFO never waits out of order.
            y_store(0, 4)
            y_store(1, 4)
            l_store(0)
            y_store(2, VP_Y[2])
            l_store(1)
            l_store(2)
            sync.wait_ge(st, 6 * 16)

        @block.scalar
        def _(scalar):
            scalar.dma_start(prm[:], p_d[:]).then_inc(ldp, 16)
            scalar.wait_ge(ldp, 16)

            for g in range(NG):
                t, _, cols = grc(g)
                scalar.wait_ge(vp, VP_ADD[g])
                nc.scalar.activation(pts[t][:, cols], yts[t][:, cols], Sigmoid,
                                     bias=prm[:, NT + t:NT + t + 1],
                                     scale=prm[:, t:t + 1]).then_inc(sp, 1)
                nc.scalar.activation(qts[t][:, cols], yts[t][:, cols], Sigmoid,
                                     bias=prm[:, 2 * NT + t:2 * NT + t + 1],
                                     scale=prm[:, t:t + 1]).then_inc(sp, 1)

        @block.vector
        def _(vector):
            def add(g):
                t, _, cols = grc(g)
                sem, need = ADD_GATE[g]
                vector.wait_ge(ldt[sem], need)
                nc.vector.tensor_tensor(yts[t][:, cols], xts[t][:, cols],
                                        nts[t][:, cols],
                                        op=op_add).then_inc(vp, 1)

            def sub_ts(t):
                # Needs all 4 acts of tile t (chunks 2t and 2t+1). The
                # subtract rounds to bf16 (rel err <= 2^-9); the 1e-9 clamp
                # then runs at 2x DVE rate on bf16.
                vector.wait_ge(sp, 4 * (t + 1))
                nc.vector.tensor_tensor(lts[t][:], pts[t][:], qts[t][:],
                                        op=op_sub)
                nc.vector.tensor_scalar(lts[t][:], lts[t][:], 1e-9, None,
                                        op0=op_max).then_inc(vp, 1)

            add(0)
            add(1)
            add(2)
            add(3)
            sub_ts(0)
            add(4)
            add(5)
            sub_ts(1)
            sub_ts(2)

    nc.compile()
    return nc


def _get_program():
    if "nc" not in _CACHE:
        _CACHE["nc"] = _build_program()
    return _CACHE["nc"]


def _build_in_maps(x, noise, ws, bs):
    """Per-core input dicts: sharded x/noise + folded per-partition params."""
    M, D = _fold_affine(ws, bs)  # (C,) float64 each, M > 0
    ch = np.arange(ROWS) // 2  # channel id per folded row
    Mr, Dr = M[ch], D[ch]
    # p/q = sigmoid(M * y + (D +- M/2)); lik = max(p - q, 1e-9)
    scl = Mr.astype(np.float32).reshape(NT, 128).T
    bpv = (Dr + Mr / 2).astype(np.float32).reshape(NT, 128).T
    bqv = (Dr - Mr / 2).astype(np.float32).reshape(NT, 128).T
    prm = np.ascontiguousarray(np.concatenate([scl, bpv, bqv], axis=1))

    x = np.ascontiguousarray(np.asarray(x, np.float32))
    noise = np.ascontiguousarray(np.asarray(noise, np.float32))
    return [
        {
            "x": x[b].reshape(ROWS, COLS),
            "noise": noise[b].reshape(ROWS, COLS),
            "prm": prm,
        }
        for b in range(NCORES)
    ]


def kernel(x, noise, w0, b0, f0, w1, b1, f1, w2, b2, f2, w3, b3):
    from concourse.bass_utils import run_bass_kernel_spmd

    ws = [w0, w1, w2, w3]
    bs = [b0, b1, b2, b3]
    fs = [f0, f1, f2]

    if any(np.any(np.asarray(f) != 0.0) for f in fs):
        # Gated (non-affine) case: bit-accurate host fallback. Never taken for
        # this module's initialization (all gates are zero).
        return _numpy_fallback(x, noise, ws, bs, fs)

    nc = _get_program()
    in_maps = _build_in_maps(x, noise, ws, bs)
    res = run_bass_kernel_spmd(nc, in_maps, list(range(NCORES))).results

    y = np.stack([np.asarray(res[b]["y"]).reshape(C, H, W) for b in range(NCORES)])
    lik = np.stack([
        np.asarray(res[b]["lik"]).astype(np.float32).reshape(C, H, W)
        for b in range(NCORES)
    ])
    return y, lik
